# revision 27
# baseline (speedup 1.0000x reference)
"""Expert-parallel MoE SwiGLU kernel for 8 Trainium2 NeuronCores.

Strategy: expert parallelism with host-side dispatch/combine. Each of the
8 cores owns one expert's weights. The host routes tokens by expert_idx,
packs each expert's tokens as a transposed [D, W] panel (features on
partitions so no on-chip transposes are needed anywhere), and each core
runs a dense SwiGLU FFN:  yT = w_down.T-blocks @ (silu(wg.T@xT) * (wu.T@xT)).
Matmul operands stream as fp16 (fp32 PSUM accumulation; ~6e-4 max
relative error vs the fp32 reference), halving the weight traffic that
dominates this memory-bound kernel. Set MOE_KERNEL_DTYPE=float32r for a
full-precision fp32 variant (~2.4e-4, ~1.5x slower).
"""

import numpy as np
from contextlib import ExitStack

D_MODEL = 1024
D_FF = 4096
N_EXPERTS = 8
N_CORES = 8

_ND = D_MODEL // 128  # 8 contraction chunks over d_model
_NF = D_FF // 128     # 32 f chunks

_nc_cache = {}

# compute dtype for matmul operands: "float32r" (safest), "float16", "bfloat16"
import os as _os
_CDT = _os.environ.get("MOE_KERNEL_DTYPE", "float16")

# f columns per gate/up weight streaming group: keep DMA lines at 2KB
_FSG = 512 if _CDT == "float32r" else 1024
_NFSG = D_FF // _FSG
_FTG = _FSG // 128    # f-tiles per group


def _np_cdt():
    if _CDT == "float16":
        return np.float16
    if _CDT == "bfloat16":
        import ml_dtypes
        return ml_dtypes.bfloat16
    return np.float32


def _build_nc(W: int):
    """Build + schedule the per-core Bass program for token capacity W."""
    import concourse.bacc as bacc
    import concourse.tile as tile
    from concourse import mybir

    f32 = mybir.dt.float32
    f32r = getattr(mybir.dt, _CDT)

    nc = bacc.Bacc("TRN2", target_bir_lowering=False, debug=False,
                   num_devices=N_CORES)
    xt = nc.dram_tensor("xt", [D_MODEL, W], f32r, kind="ExternalInput").ap()
    wg = nc.dram_tensor("wg", [_NFSG, _ND, 128, _FSG], f32r,
                        kind="ExternalInput").ap()
    wu = nc.dram_tensor("wu", [_NFSG, _ND, 128, _FSG], f32r,
                        kind="ExternalInput").ap()
    wd = nc.dram_tensor("wd", [D_FF, D_MODEL], f32r, kind="ExternalInput").ap()
    yt = nc.dram_tensor("yt", [D_MODEL, W], f32, kind="ExternalOutput").ap()

    with tile.TileContext(nc) as tc, ExitStack() as ctx:
        xpool = ctx.enter_context(tc.tile_pool(name="x", bufs=1))
        wgp = ctx.enter_context(tc.tile_pool(name="wgp", bufs=4))
        wup = ctx.enter_context(tc.tile_pool(name="wup", bufs=4))
        wdp = ctx.enter_context(tc.tile_pool(name="wdp", bufs=3))
        tp = ctx.enter_context(tc.tile_pool(name="tp", bufs=2))
        gap = ctx.enter_context(tc.tile_pool(name="gap", bufs=3))
        yp = ctx.enter_context(tc.tile_pool(name="yp", bufs=1))
        pg = ctx.enter_context(tc.tile_pool(name="pg", bufs=2, space="PSUM"))
        pu = ctx.enter_context(tc.tile_pool(name="pu", bufs=2, space="PSUM"))
        pd = ctx.enter_context(tc.tile_pool(name="pd", bufs=4, space="PSUM"))

        # Input activations, transposed: d_model on partitions.
        xts = []
        for d in range(_ND):
            x_t = xpool.tile([128, W], f32r, tag=f"x{d}")
            nc.scalar.dma_start(x_t[:], xt[d * 128:(d + 1) * 128, :])
            xts.append(x_t)

        y_acc = [yp.tile([128, W], f32, tag=f"y{d}", name=f"y_acc{d}")
                 for d in range(_ND)]

        # HAM warm-up scratch: dummy matmuls interleaved through f group 0
        # keep the PE activity monitor busy while real weights stream in,
        # so real matmuls run at 2.4GHz instead of the cold 1.2GHz.
        scr_w = xpool.tile([128, 128], f32r, tag="scrw", name="scr_w")
        scr_x = xpool.tile([128, W], f32r, tag="scrx", name="scr_x")
        nc.vector.memset(scr_w[:], 0.0)
        nc.vector.memset(scr_x[:], 0.0)
        scr_p = pd.tile([128, W], f32, tag="pd", name="scr_p")
        scr_p2 = pd.tile([128, W], f32, tag="pd", name="scr_p2")
        _scr = [scr_p, scr_p2]

        def emit_warmup(n):
            for i in range(n):
                nc.tensor.matmul(_scr[i % 2][:], scr_w[:], scr_x[:],
                                 start=True, stop=True)

        # dense opening burst: ~3.6us of continuous PE activity while the
        # first weight tiles are still in flight trips the HAM busy window
        # before the first real matmul, so everything runs at 2.4GHz.
        emit_warmup(8)

        # Fused pipeline over f groups: gate/up matmuls + SwiGLU produce
        # short-lived t tiles; the down-projection of the PREVIOUS f group
        # is interleaved between this group's matmul bursts so the PE's
        # DMA-wait gaps are broken into sub-HAM-window slices. Weight DMA
        # issue is split across both HWDGE rings (sync + scalar engines).
        def emit_down(fsg, t_tiles, wd_tiles, dts):
            # y[dt] += wd[fgroup rows, dt cols].T @ t   for dt in dts
            for dt in dts:
                pdt = pd.tile([128, W], f32, tag="pd", name=f"pd_{fsg}_{dt}")
                for ft in range(_FTG):
                    nc.tensor.matmul(
                        pdt[:],
                        wd_tiles[ft][:, dt * 128:(dt + 1) * 128],
                        t_tiles[ft][:],
                        start=(ft == 0), stop=(ft == _FTG - 1))
                if fsg == 0:
                    nc.vector.tensor_copy(y_acc[dt][:], pdt[:])
                else:
                    nc.vector.tensor_add(y_acc[dt][:], y_acc[dt][:], pdt[:])

        prev = None  # (fsg, t_tiles, wd_tiles) of the previous f group
        for fsg in range(_NFSG):
            wg_t, wu_t = [], []
            if fsg == 0:
                # gate tiles first: the first matmul group needs all 8
                for d in range(_ND):
                    g_t = wgp.tile([128, _FSG], f32r, tag=f"wg{d}")
                    nc.sync.dma_start(g_t[:], wg[fsg, d])
                    wg_t.append(g_t)
                for d in range(_ND):
                    u_t = wup.tile([128, _FSG], f32r, tag=f"wu{d}")
                    nc.sync.dma_start(u_t[:], wu[fsg, d])
                    wu_t.append(u_t)
            else:
                for d in range(_ND):
                    g_t = wgp.tile([128, _FSG], f32r, tag=f"wg{d}")
                    nc.sync.dma_start(g_t[:], wg[fsg, d])
                    wg_t.append(g_t)
                    u_t = wup.tile([128, _FSG], f32r, tag=f"wu{d}")
                    nc.sync.dma_start(u_t[:], wu[fsg, d])
                    wu_t.append(u_t)

            def g_slice(d, ft):
                return wg_t[d][:, ft * 128:(ft + 1) * 128]

            def u_slice(d, ft):
                return wu_t[d][:, ft * 128:(ft + 1) * 128]

            t_tiles = []
            wd_tiles = []
            for ft in range(_FTG):
                fc = fsg * _FTG + ft
                wd_t = wdp.tile([128, D_MODEL], f32r, tag=f"wd{ft}")
                nc.sync.dma_start(wd_t[:], wd[fc * 128:(fc + 1) * 128, :])
                wd_tiles.append(wd_t)
                if fsg == 0 and ft < 4:
                    emit_warmup(2)
                psg = pg.tile([128, W], f32)
                for d in range(_ND):
                    nc.tensor.matmul(
                        psg[:],
                        g_slice(d, ft),
                        xts[d][:],
                        start=(d == 0), stop=(d == _ND - 1))
                if fsg == 0 and ft < 4:
                    emit_warmup(2)
                psu = pu.tile([128, W], f32)
                for d in range(_ND):
                    nc.tensor.matmul(
                        psu[:],
                        u_slice(d, ft),
                        xts[d][:],
                        start=(d == 0), stop=(d == _ND - 1))
                g_act = gap.tile([128, W], f32, tag="gact")
                nc.scalar.activation(g_act[:], psg[:],
                                     mybir.ActivationFunctionType.Silu)
                t_t = tp.tile([128, W], f32r, tag=f"t{ft}")
                nc.vector.tensor_mul(t_t[:], g_act[:], psu[:])
                t_tiles.append(t_t)
                if prev is not None:
                    if _FTG == 8:
                        emit_down(prev[0], prev[1], prev[2], (ft,))
                    else:
                        emit_down(prev[0], prev[1], prev[2], (2 * ft, 2 * ft + 1))
            prev = (fsg, t_tiles, wd_tiles)
        emit_down(prev[0], prev[1], prev[2], range(_ND))

        for dt in range(_ND):
            nc.sync.dma_start(yt[dt * 128:(dt + 1) * 128, :], y_acc[dt][:])

    nc.compile()
    return nc


def _pack_gu(w):
    # [D, F] -> [NFSG, ND, 128, FSG] so each streamed tile is contiguous
    w = np.asarray(w).astype(_np_cdt())
    return np.ascontiguousarray(
        w.reshape(_ND, 128, _NFSG, _FSG).transpose(2, 0, 1, 3))


def _run_one(W, tok_lists, x_flat, packed_w, out_flat):
    from concourse.bass_utils import run_bass_kernel_spmd

    if W not in _nc_cache:
        _nc_cache[W] = _build_nc(W)
    nc = _nc_cache[W]

    D = x_flat.shape[1]
    in_maps = []
    for e in range(N_EXPERTS):
        toks = tok_lists[e]
        xt_e = np.zeros((D, W), dtype=_np_cdt())
        xt_e[:, :len(toks)] = x_flat[toks].T.astype(_np_cdt())
        in_maps.append({
            "xt": xt_e,
            "wg": packed_w[e][0],
            "wu": packed_w[e][1],
            "wd": packed_w[e][2],
        })

    res = None
    for attempt in range(3):
        try:
            res = run_bass_kernel_spmd(nc, in_maps,
                                       core_ids=list(range(N_CORES)))
            break
        except Exception:
            if attempt == 2:
                raise
            import time
            time.sleep(2.0)
    for e in range(N_EXPERTS):
        toks = tok_lists[e]
        out_flat[toks] = res.results[e]["yt"][:, :len(toks)].T


def kernel(x, expert_idx, w_gate, w_up, w_down):
    x = np.asarray(x, dtype=np.float32)
    idx = np.asarray(expert_idx).astype(np.int64)
    B, S, D = x.shape
    T = B * S
    x_flat = np.ascontiguousarray(x.reshape(T, D))
    idx_flat = idx.reshape(T)

    packed_w = [
        (_pack_gu(w_gate[e]), _pack_gu(w_up[e]),
         np.ascontiguousarray(np.asarray(w_down[e]).astype(_np_cdt())))
        for e in range(N_EXPERTS)
    ]

    tok_lists = [np.nonzero(idx_flat == e)[0] for e in range(N_EXPERTS)]
    cap = max(1, max(len(t) for t in tok_lists))
    out_flat = np.zeros((T, D), dtype=np.float32)

    if cap <= 512:
        # normal path: one SPMD run, capacity = max expert load (floor 256
        # keeps DMA partition lines >= 512B)
        W = max(256, cap)
        _run_one(W, tok_lists, x_flat, packed_w, out_flat)
    else:
        # fallback for extreme routing imbalance: process tokens in
        # rounds of <=512 per expert, reusing one compiled W=512 program
        rounds = -(-cap // 512)
        for r in range(rounds):
            round_lists = [t[r * 512:(r + 1) * 512] for t in tok_lists]
            _run_one(512, round_lists, x_flat, packed_w, out_flat)

    return out_flat.reshape(B, S, D)


# revision 28
# speedup vs baseline: 1.0024x; 1.0024x over previous
"""Expert-parallel MoE SwiGLU kernel for 8 Trainium2 NeuronCores.

Strategy: expert parallelism with host-side dispatch/combine. Each of the
8 cores owns one expert's weights. The host routes tokens by expert_idx,
packs each expert's tokens as a transposed [D, W] panel (features on
partitions so no on-chip transposes are needed anywhere), and each core
runs a dense SwiGLU FFN:  yT = w_down.T-blocks @ (silu(wg.T@xT) * (wu.T@xT)).
Matmul operands stream as fp16 (fp32 PSUM accumulation; ~6e-4 max
relative error vs the fp32 reference), halving the weight traffic that
dominates this memory-bound kernel. Set MOE_KERNEL_DTYPE=float32r for a
full-precision fp32 variant (~2.4e-4, ~1.5x slower).
"""

import numpy as np
from contextlib import ExitStack

D_MODEL = 1024
D_FF = 4096
N_EXPERTS = 8
N_CORES = 8

_ND = D_MODEL // 128  # 8 contraction chunks over d_model
_NF = D_FF // 128     # 32 f chunks

_nc_cache = {}

# compute dtype for matmul operands: "float32r" (safest), "float16", "bfloat16"
import os as _os
_CDT = _os.environ.get("MOE_KERNEL_DTYPE", "float16")

# f columns per gate/up weight streaming group: keep DMA lines at 2KB
_FSG = 512 if _CDT == "float32r" else 1024
_NFSG = D_FF // _FSG
_FTG = _FSG // 128    # f-tiles per group


def _np_cdt():
    if _CDT == "float16":
        return np.float16
    if _CDT == "bfloat16":
        import ml_dtypes
        return ml_dtypes.bfloat16
    return np.float32


def _build_nc(W: int):
    """Build + schedule the per-core Bass program for token capacity W."""
    import concourse.bacc as bacc
    import concourse.tile as tile
    from concourse import mybir

    f32 = mybir.dt.float32
    f32r = getattr(mybir.dt, _CDT)

    nc = bacc.Bacc("TRN2", target_bir_lowering=False, debug=False,
                   num_devices=N_CORES)
    xt = nc.dram_tensor("xt", [D_MODEL, W], f32r, kind="ExternalInput").ap()
    wg = nc.dram_tensor("wg", [_NFSG, _ND, 128, _FSG], f32r,
                        kind="ExternalInput").ap()
    wu = nc.dram_tensor("wu", [_NFSG, _ND, 128, _FSG], f32r,
                        kind="ExternalInput").ap()
    wd = nc.dram_tensor("wd", [D_FF, D_MODEL], f32r, kind="ExternalInput").ap()
    yt = nc.dram_tensor("yt", [D_MODEL, W], f32, kind="ExternalOutput").ap()

    with tile.TileContext(nc) as tc, ExitStack() as ctx:
        xpool = ctx.enter_context(tc.tile_pool(name="x", bufs=1))
        wgp = ctx.enter_context(tc.tile_pool(name="wgp", bufs=4))
        wup = ctx.enter_context(tc.tile_pool(name="wup", bufs=4))
        wdp = ctx.enter_context(tc.tile_pool(name="wdp", bufs=3))
        tp = ctx.enter_context(tc.tile_pool(name="tp", bufs=2))
        gap = ctx.enter_context(tc.tile_pool(name="gap", bufs=3))
        yp = ctx.enter_context(tc.tile_pool(name="yp", bufs=1))
        pg = ctx.enter_context(tc.tile_pool(name="pg", bufs=2, space="PSUM"))
        pu = ctx.enter_context(tc.tile_pool(name="pu", bufs=2, space="PSUM"))
        pd = ctx.enter_context(tc.tile_pool(name="pd", bufs=4, space="PSUM"))

        # Input activations, transposed: d_model on partitions.
        xts = []
        for d in range(_ND):
            x_t = xpool.tile([128, W], f32r, tag=f"x{d}")
            nc.scalar.dma_start(x_t[:], xt[d * 128:(d + 1) * 128, :])
            xts.append(x_t)

        y_acc = [yp.tile([128, W], f32, tag=f"y{d}", name=f"y_acc{d}")
                 for d in range(_ND)]

        # HAM warm-up scratch: dummy matmuls interleaved through f group 0
        # keep the PE activity monitor busy while real weights stream in,
        # so real matmuls run at 2.4GHz instead of the cold 1.2GHz.
        scr_w = xpool.tile([128, 128], f32r, tag="scrw", name="scr_w")
        scr_x = xpool.tile([128, W], f32r, tag="scrx", name="scr_x")
        nc.vector.memset(scr_w[:], 0.0)
        nc.vector.memset(scr_x[:], 0.0)
        scr_p = pd.tile([128, W], f32, tag="pd", name="scr_p")
        scr_p2 = pd.tile([128, W], f32, tag="pd", name="scr_p2")
        _scr = [scr_p, scr_p2]

        def emit_warmup(n):
            for i in range(n):
                nc.tensor.matmul(_scr[i % 2][:], scr_w[:], scr_x[:],
                                 start=True, stop=True)

        # dense opening burst: ~3.6us of continuous PE activity while the
        # first weight tiles are still in flight trips the HAM busy window
        # before the first real matmul, so everything runs at 2.4GHz.
        emit_warmup(16)

        # Fused pipeline over f groups: gate/up matmuls + SwiGLU produce
        # short-lived t tiles; the down-projection of the PREVIOUS f group
        # is interleaved between this group's matmul bursts so the PE's
        # DMA-wait gaps are broken into sub-HAM-window slices. Weight DMA
        # issue is split across both HWDGE rings (sync + scalar engines).
        def emit_down(fsg, t_tiles, wd_tiles, dts):
            # y[dt] += wd[fgroup rows, dt cols].T @ t   for dt in dts
            for dt in dts:
                pdt = pd.tile([128, W], f32, tag="pd", name=f"pd_{fsg}_{dt}")
                for ft in range(_FTG):
                    nc.tensor.matmul(
                        pdt[:],
                        wd_tiles[ft][:, dt * 128:(dt + 1) * 128],
                        t_tiles[ft][:],
                        start=(ft == 0), stop=(ft == _FTG - 1))
                if fsg == 0:
                    nc.vector.tensor_copy(y_acc[dt][:], pdt[:])
                else:
                    nc.vector.tensor_add(y_acc[dt][:], y_acc[dt][:], pdt[:])

        prev = None  # (fsg, t_tiles, wd_tiles) of the previous f group
        for fsg in range(_NFSG):
            wg_t, wu_t = [], []
            if fsg == 0:
                # gate tiles first: the first matmul group needs all 8
                for d in range(_ND):
                    g_t = wgp.tile([128, _FSG], f32r, tag=f"wg{d}")
                    nc.sync.dma_start(g_t[:], wg[fsg, d])
                    wg_t.append(g_t)
                for d in range(_ND):
                    u_t = wup.tile([128, _FSG], f32r, tag=f"wu{d}")
                    nc.sync.dma_start(u_t[:], wu[fsg, d])
                    wu_t.append(u_t)
            else:
                for d in range(_ND):
                    g_t = wgp.tile([128, _FSG], f32r, tag=f"wg{d}")
                    nc.sync.dma_start(g_t[:], wg[fsg, d])
                    wg_t.append(g_t)
                    u_t = wup.tile([128, _FSG], f32r, tag=f"wu{d}")
                    nc.sync.dma_start(u_t[:], wu[fsg, d])
                    wu_t.append(u_t)

            def g_slice(d, ft):
                return wg_t[d][:, ft * 128:(ft + 1) * 128]

            def u_slice(d, ft):
                return wu_t[d][:, ft * 128:(ft + 1) * 128]

            t_tiles = []
            wd_tiles = []
            for ft in range(_FTG):
                fc = fsg * _FTG + ft
                wd_t = wdp.tile([128, D_MODEL], f32r, tag=f"wd{ft}")
                nc.sync.dma_start(wd_t[:], wd[fc * 128:(fc + 1) * 128, :])
                wd_tiles.append(wd_t)
                if fsg == 0 and ft < 4:
                    emit_warmup(2)
                psg = pg.tile([128, W], f32)
                for d in range(_ND):
                    nc.tensor.matmul(
                        psg[:],
                        g_slice(d, ft),
                        xts[d][:],
                        start=(d == 0), stop=(d == _ND - 1))
                if fsg == 0 and ft < 4:
                    emit_warmup(2)
                psu = pu.tile([128, W], f32)
                for d in range(_ND):
                    nc.tensor.matmul(
                        psu[:],
                        u_slice(d, ft),
                        xts[d][:],
                        start=(d == 0), stop=(d == _ND - 1))
                g_act = gap.tile([128, W], f32, tag="gact")
                nc.scalar.activation(g_act[:], psg[:],
                                     mybir.ActivationFunctionType.Silu)
                t_t = tp.tile([128, W], f32r, tag=f"t{ft}")
                nc.vector.tensor_mul(t_t[:], g_act[:], psu[:])
                t_tiles.append(t_t)
                if prev is not None:
                    if _FTG == 8:
                        emit_down(prev[0], prev[1], prev[2], (ft,))
                    else:
                        emit_down(prev[0], prev[1], prev[2], (2 * ft, 2 * ft + 1))
            prev = (fsg, t_tiles, wd_tiles)
        emit_down(prev[0], prev[1], prev[2], range(_ND))

        for dt in range(_ND):
            nc.sync.dma_start(yt[dt * 128:(dt + 1) * 128, :], y_acc[dt][:])

    nc.compile()
    return nc


def _pack_gu(w):
    # [D, F] -> [NFSG, ND, 128, FSG] so each streamed tile is contiguous
    w = np.asarray(w).astype(_np_cdt())
    return np.ascontiguousarray(
        w.reshape(_ND, 128, _NFSG, _FSG).transpose(2, 0, 1, 3))


def _run_one(W, tok_lists, x_flat, packed_w, out_flat):
    from concourse.bass_utils import run_bass_kernel_spmd

    if W not in _nc_cache:
        _nc_cache[W] = _build_nc(W)
    nc = _nc_cache[W]

    D = x_flat.shape[1]
    in_maps = []
    for e in range(N_EXPERTS):
        toks = tok_lists[e]
        xt_e = np.zeros((D, W), dtype=_np_cdt())
        xt_e[:, :len(toks)] = x_flat[toks].T.astype(_np_cdt())
        in_maps.append({
            "xt": xt_e,
            "wg": packed_w[e][0],
            "wu": packed_w[e][1],
            "wd": packed_w[e][2],
        })

    res = None
    for attempt in range(3):
        try:
            res = run_bass_kernel_spmd(nc, in_maps,
                                       core_ids=list(range(N_CORES)))
            break
        except Exception:
            if attempt == 2:
                raise
            import time
            time.sleep(2.0)
    for e in range(N_EXPERTS):
        toks = tok_lists[e]
        out_flat[toks] = res.results[e]["yt"][:, :len(toks)].T


def kernel(x, expert_idx, w_gate, w_up, w_down):
    x = np.asarray(x, dtype=np.float32)
    idx = np.asarray(expert_idx).astype(np.int64)
    B, S, D = x.shape
    T = B * S
    x_flat = np.ascontiguousarray(x.reshape(T, D))
    idx_flat = idx.reshape(T)

    packed_w = [
        (_pack_gu(w_gate[e]), _pack_gu(w_up[e]),
         np.ascontiguousarray(np.asarray(w_down[e]).astype(_np_cdt())))
        for e in range(N_EXPERTS)
    ]

    tok_lists = [np.nonzero(idx_flat == e)[0] for e in range(N_EXPERTS)]
    cap = max(1, max(len(t) for t in tok_lists))
    out_flat = np.zeros((T, D), dtype=np.float32)

    if cap <= 512:
        # normal path: one SPMD run, capacity = max expert load (floor 256
        # keeps DMA partition lines >= 512B)
        W = max(256, cap)
        _run_one(W, tok_lists, x_flat, packed_w, out_flat)
    else:
        # fallback for extreme routing imbalance: process tokens in
        # rounds of <=512 per expert, reusing one compiled W=512 program
        rounds = -(-cap // 512)
        for r in range(rounds):
            round_lists = [t[r * 512:(r + 1) * 512] for t in tok_lists]
            _run_one(512, round_lists, x_flat, packed_w, out_flat)

    return out_flat.reshape(B, S, D)
